# revision 6
# baseline (speedup 1.0000x reference)
"""EGConv + GraphNorm + ReLU Trainium2 kernel (8 NeuronCores, SPMD).

Strategy (hardcoded for N=100000, E=3200000, D=128, H=8, B=4, A=['sum','max'],
G=64 graphs):
  - Nodes partitioned across 8 cores at graph boundaries (GraphNorm stays
    core-local). Each core owns its dst nodes and their incident edges.
  - Edges gathered per-dst via SWDGE dma_gather (int16 indices -> the padded
    global bases table is split into 4 chunks of 2*NMAX <= 32768 rows; the
    gather source AP per call is one chunk).
  - Per-core dst nodes are sorted by their per-chunk in-degree vector so that
    128-node tiles have tight per-chunk max degrees (less padding).
  - The gather table holds bases + BIG (BIG=512) for real rows and 0 for pad
    rows; padding slots point at pad rows, so
        segment_sum = reduce_sum - k_dst*BIG,   segment_max = reduce_max - BIG.
  - comb/bases matmuls, GraphNorm segment stats (via indicator matmuls) and
    the per-graph affine run on TensorE; reductions and the (comb x aggr)
    einsum run on VectorE/GpSimd.
"""

import math
import os
import numpy as np

# ---------------- problem constants (hardcoded per spec) ----------------
N = 100000
E = 3200000
D = 128
H = 8
NB = 4          # num bases
FH = 16         # per-head dim
G = 64          # graphs
EPS = 1e-5
P = 128
NCORES = 8
BF = 64         # bases feature dim = NB*FH
BIG = 512.0
GPAD = 16       # padded per-core graph count
COLCAP = 8      # max gather columns per call (8*128 = 1024 descriptors)
SLOTCAP = 128   # max T*Wtot slots per supertile (SBUF budget)
TMAX = 1

_F32 = np.float32


def _ceil(a, b):
    return -(-a // b) * b


# ======================================================================
# host preprocessing
# ======================================================================
def _prep(edge_index, batch_ptr):
    counts = np.bincount(batch_ptr, minlength=G).astype(np.int64)
    gcum = np.concatenate([[0], np.cumsum(counts)])  # [G+1]

    # core boundaries at graph boundaries, close to N/8 multiples
    gb = [0]
    for c in range(1, NCORES):
        tgt = N * c / NCORES
        g = int(np.argmin(np.abs(gcum - tgt)))
        g = min(max(g, gb[-1]), G - (NCORES - c))
        gb.append(g)
    gb.append(G)
    node0 = np.array([gcum[gb[c]] for c in range(NCORES)], np.int64)
    ncs = np.array([gcum[gb[c + 1]] - gcum[gb[c]] for c in range(NCORES)],
                   np.int64)
    NMAX = _ceil(int(ncs.max()) + 1, P)
    assert 2 * NMAX <= 32768, (NMAX, ncs.max())
    CHUNK = 2 * NMAX
    ntiles = NMAX // P

    src_g = np.asarray(edge_index[0], np.int64)
    dst_g = np.asarray(edge_index[1], np.int64)
    bounds = np.concatenate([node0, [N]])
    node_core = np.searchsorted(bounds, np.arange(N), side="right") - 1
    node_local = np.arange(N) - node0[node_core]

    src_core = node_core[src_g]
    dst_core = node_core[dst_g]

    cores = []
    for c in range(NCORES):
        n_c = int(ncs[c])
        m = dst_core == c
        es = src_g[m]
        ed = dst_g[m] - node0[c]
        # self loops
        own = np.arange(n_c, dtype=np.int64)
        es = np.concatenate([es, own + node0[c]])
        ed = np.concatenate([ed, own])
        ch = node_core[es] >> 1  # chunk of each edge's src

        # per-(dst, chunk) counts
        kmat = np.bincount(ed * 4 + ch, minlength=n_c * 4).reshape(n_c, 4)
        # sort desc by (max_c k_c, k0, k1, k2, k3) — tight per-tile windows
        kmx = kmat.max(axis=1)
        order = np.lexsort((-kmat[:, 3], -kmat[:, 2], -kmat[:, 1],
                            -kmat[:, 0], -kmx))
        pos_of = np.empty(n_c, np.int64)
        pos_of[order] = np.arange(n_c)
        cores.append(dict(n=n_c, node0=int(node0[c]), perm=order,
                          pos_of=pos_of, es=es, ed=ed, ch=ch, kmat=kmat,
                          g0=gb[c], g1=gb[c + 1]))

    # pid of a global node id: core*NMAX + sorted position
    pid_of = np.empty(N, np.int64)
    for c in range(NCORES):
        cc = cores[c]
        pid_of[cc["node0"]:cc["node0"] + cc["n"]] = c * NMAX + cc["pos_of"]

    # per-core sorted-space per-chunk CSR + k arrays
    for c in range(NCORES):
        cc = cores[c]
        dpos = cc["pos_of"][cc["ed"]]
        key = dpos * 4 + cc["ch"]
        eorder = np.argsort(key, kind="stable")
        rel = (pid_of[cc["es"]] - cc["ch"] * CHUNK)[eorder]
        assert rel.min() >= 0 and rel.max() < CHUNK
        kflat = np.bincount(key, minlength=NMAX * 4)
        indptr = np.concatenate([[0], np.cumsum(kflat)])
        ks = kflat.reshape(NMAX, 4)  # sorted space, dummies are 0
        cc["csr_data"] = rel.astype(np.int64)
        cc["csr_ptr"] = indptr
        cc["ks"] = ks

    # zero-row (pad row) per chunk: core 2c's first pad row
    zrow_rel = np.array([cores[2 * c]["n"] for c in range(4)], np.int64)

    # shared per-tile per-chunk window widths (max over cores)
    Wct = np.zeros((ntiles, 4), np.int64)
    for c in range(NCORES):
        km = cores[c]["ks"].reshape(ntiles, P, 4)
        Wct = np.maximum(Wct, km.max(axis=1))

    # supertile schedule
    sched = []
    t = 0
    while t < ntiles:
        T = 1
        while T < TMAX and t + T < ntiles:
            wc = Wct[t:t + T + 1].max(axis=0)
            if (T + 1) * int(wc.sum()) > SLOTCAP:
                break
            T += 1
        wc = Wct[t:t + T].max(axis=0)
        wtot = int(wc.sum())
        if wtot == 0:
            wtot = 1  # degenerate; shouldn't happen (self loops)
        coff = np.concatenate([[0], np.cumsum(wc)])
        calls = []  # (chunk, tile_rel, col0_in_msg, ncols, s16_off)
        s16 = 0
        for tr in range(T):
            for chx in range(4):
                w = int(wc[chx])
                done = 0
                while done < w:
                    nc_ = min(COLCAP, w - done)
                    calls.append((chx, tr, int(coff[chx]) + done, nc_, s16))
                    s16 += nc_ * 8
                    done += nc_
        sched.append(dict(t0=t, T=T, wc=wc.copy(), wtot=wtot,
                          coff=coff.copy(), calls=calls, s16=s16))
        t += T
    S16TOT = sum(s["s16"] for s in sched)

    # per-core device input arrays
    for c in range(NCORES):
        cc = cores[c]
        data, ptr, ks = cc["csr_data"], cc["csr_ptr"], cc["ks"]
        idx16 = np.zeros((P, S16TOT), np.int16)
        s16base = 0
        for s in sched:
            for (chx, tr, col0, ncols, s16off) in s["calls"]:
                tt = s["t0"] + tr
                dp = tt * P + np.arange(P)
                cnt = ks[dp, chx]
                start = ptr[dp * 4 + chx]
                j0 = col0 - int(s["coff"][chx])
                jj = j0 + np.arange(ncols)[None, :]
                gidx = np.minimum(start[:, None] + jj,
                                  max(len(data) - 1, 0))
                vals = data[gidx] if len(data) else np.zeros((P, ncols),
                                                             np.int64)
                mat = np.where(jj < cnt[:, None], vals, zrow_rel[chx])
                flat = mat.T.reshape(-1)  # [ncols*128], i = col*128 + p
                wrapped = flat.reshape(-1, 16).T.astype(np.int16)  # [16, S]
                idx16[:, s16base + s16off:
                      s16base + s16off + ncols * 8] = np.tile(wrapped, (8, 1))
            s16base += s["s16"]
        cc["idx16"] = idx16

        ktot = ks.sum(axis=1).astype(_F32)  # [NMAX] sorted space
        cc["kbig"] = (ktot.reshape(ntiles, P).T * BIG).astype(_F32)  # [P,nt]
        real = (np.arange(NMAX) < cc["n"])
        cc["shift"] = np.where(real.reshape(ntiles, P).T, _F32(BIG),
                               _F32(0.0)).astype(_F32)

        # graph id per sorted position
        gid = np.full(NMAX, -1, np.int64)
        gnode = np.searchsorted(gcum, cc["node0"] + cc["perm"],
                                side="right") - 1 - cc["g0"]
        gid[:cc["n"]] = gnode
        ind = np.zeros((P, ntiles * GPAD), _F32)
        indT = np.zeros((GPAD, ntiles * P), _F32)
        for tt in range(ntiles):
            gl = gid[tt * P:(tt + 1) * P]
            valid = gl >= 0
            pidx = np.arange(P)[valid]
            gv = gl[valid]
            ind[pidx, tt * GPAD + gv] = 1.0
            indT[gv, tt * P + pidx] = 1.0
        cc["ind"] = ind
        cc["indT"] = indT
        cc["gid"] = gid

        cnt_loc = counts[cc["g0"]:cc["g1"]].astype(_F32)
        cntinv = np.zeros(GPAD, _F32)
        cntinv[:len(cnt_loc)] = 1.0 / np.maximum(cnt_loc, 1.0)
        cc["cntinv"] = cntinv

    return dict(cores=cores, NMAX=NMAX, CHUNK=CHUNK, ntiles=ntiles,
                sched=sched, S16TOT=S16TOT, zrow_rel=zrow_rel,
                node0=node0, ncs=ncs)


def _make_inputs(cfg, node, W_bases, W_comb, b_comb, bias_out, gn_weight,
                 gn_bias, gn_mean_scale):
    node = np.asarray(node, _F32)
    NMAX, ntiles = cfg["NMAX"], cfg["ntiles"]
    wcat = np.concatenate([np.asarray(W_bases, _F32),
                           np.asarray(W_comb, _F32)], axis=1)  # [128,128]
    bcomb = np.asarray(b_comb, _F32).reshape(1, BF)
    gaux = np.zeros((GPAD, 520), _F32)
    gaux[:, 1:129] = np.asarray(bias_out, _F32)[None, :]
    gaux[:, 129:257] = np.asarray(gn_mean_scale, _F32)[None, :]
    gaux[:, 257:385] = np.asarray(gn_weight, _F32)[None, :]
    gaux[:, 385:513] = np.asarray(gn_bias, _F32)[None, :]

    in_maps = []
    for c in range(NCORES):
        cc = cfg["cores"][c]
        nperm = np.zeros((NMAX, D), _F32)
        nperm[:cc["n"]] = node[cc["node0"]:cc["node0"] + cc["n"]][cc["perm"]]
        ga = gaux.copy()
        ga[:, 0] = cc["cntinv"]
        in_maps.append({
            "nodeT": np.ascontiguousarray(nperm.T),        # [128, NMAX]
            "wcat": wcat,
            "bcomb": bcomb,
            "idx": cc["idx16"],                            # [128, S16TOT]
            "kbig": np.ascontiguousarray(cc["kbig"]),      # [128, ntiles]
            "shift": np.ascontiguousarray(cc["shift"]),    # [128, ntiles]
            "ind": np.ascontiguousarray(
                cc["ind"].astype(np.dtype("bfloat16")
                                 if False else _F32)),     # [128, nt*16]
            "indT": np.ascontiguousarray(cc["indT"]),      # [16, nt*128]
            "gaux": ga,                                    # [16, 520]
        })
    return in_maps


# ======================================================================
# numpy simulation of the device algorithm (bit-approximate, for testing)
# ======================================================================
def _numpy_sim(cfg, in_maps):
    NMAX, CHUNK, ntiles = cfg["NMAX"], cfg["CHUNK"], cfg["ntiles"]
    # phase A+B: bases table (shared), per-core comb
    table = np.zeros((NCORES * NMAX, BF), _F32)
    combs = []
    for c in range(NCORES):
        im = in_maps[c]
        nodeT = im["nodeT"]
        full = nodeT.T @ im["wcat"]  # [NMAX, 128]
        bases = full[:, :BF]
        comb = full[:, BF:] + im["bcomb"][0][None, :]
        shift = im["shift"].T.reshape(-1)  # [NMAX]
        table[c * NMAX:(c + 1) * NMAX] = bases + shift[:, None]
        combs.append(comb)

    outs = []
    for c in range(NCORES):
        im = in_maps[c]
        h0 = np.zeros((NMAX, D), _F32)
        s16base = 0
        kbig = im["kbig"].T  # [ntiles, 128]
        for s in cfg["sched"]:
            T, wtot = s["T"], s["wtot"]
            msg = np.zeros((P, T, wtot, BF), _F32)
            for (chx, tr, col0, ncols, s16off) in s["calls"]:
                blk = im["idx"][:16, s16base + s16off:
                                s16base + s16off + ncols * 8]
                # unwrap: value[i] = blk[i % 16, i // 16]
                f2 = blk.T.reshape(-1)  # order (s, p): f2[s*16+p] = blk[p, s]
                vals = f2[:ncols * 128].astype(np.int64)
                rows = table[chx * CHUNK + vals.reshape(ncols, P)]
                msg[:, tr, col0:col0 + ncols, :] = rows.transpose(1, 0, 2)
            s16base += s["s16"]
            ssum = msg.sum(axis=2)                    # [P, T, 64]
            smax = msg.max(axis=2) - _F32(BIG)        # [P, T, 64]
            for tr in range(s["T"]):
                tt = s["t0"] + tr
                su = ssum[:, tr, :] - kbig[tt][:, None]
                aggcat = np.concatenate([su, smax[:, tr, :]], axis=1)
                comb = combs[c][tt * P:(tt + 1) * P]  # [128, 64]
                prod = (comb.reshape(P, H, 8, 1) *
                        aggcat.reshape(P, 1, 8, FH))
                h0[tt * P:(tt + 1) * P] = prod.sum(axis=2).reshape(P, D)
        # graphnorm
        ind = im["ind"].reshape(P, ntiles, GPAD)
        ga = im["gaux"]
        cntinv = ga[:, 0:1]
        bias_o = ga[:, 1:129]
        ms = ga[:, 129:257]
        gnw = ga[:, 257:385]
        gnb = ga[:, 385:513]
        s1 = np.zeros((GPAD, D), _F32)
        s2 = np.zeros((GPAD, D), _F32)
        for tt in range(ntiles):
            ht = h0[tt * P:(tt + 1) * P]
            s1 += ind[:, tt, :].T @ ht
            s2 += ind[:, tt, :].T @ (ht * ht)
        m0 = s1 * cntinv
        mh = m0 + bias_o
        e2 = s2 * cntinv + bias_o * (2 * m0 + bias_o)
        c0 = mh * ms
        var = e2 - 2 * c0 * mh + c0 * c0
        rstd = 1.0 / np.sqrt(var + EPS)
        Pm = gnw * rstd
        Qm = (bias_o - c0) * Pm + gnb
        indT = im["indT"].reshape(GPAD, ntiles, P)
        hfin = np.zeros((NMAX, D), _F32)
        for tt in range(ntiles):
            Pn = indT[:, tt, :].T @ Pm
            Qn = indT[:, tt, :].T @ Qm
            hfin[tt * P:(tt + 1) * P] = np.maximum(
                h0[tt * P:(tt + 1) * P] * Pn + Qn, 0.0)
        outs.append(hfin)
    return outs


def _assemble(cfg, per_core_h):
    out = np.zeros((N, D), _F32)
    for c in range(NCORES):
        cc = cfg["cores"][c]
        out[cc["node0"] + cc["perm"]] = per_core_h[c][:cc["n"]]
    return out


# ======================================================================
# device program
# ======================================================================
def _build(cfg):
    import concourse.bacc as bacc
    import concourse.tile as tile
    from concourse import mybir

    NMAX, CHUNK, ntiles = cfg["NMAX"], cfg["CHUNK"], cfg["ntiles"]
    S16TOT = cfg["S16TOT"]
    f32 = mybir.dt.float32
    bf16 = mybir.dt.bfloat16
    ALU = mybir.AluOpType
    ACT = mybir.ActivationFunctionType
    AX = mybir.AxisListType

    nc = bacc.Bacc("TRN2", target_bir_lowering=False, debug=False,
                   num_devices=NCORES, num_swdge_queues=4)

    nodeT = nc.dram_tensor("nodeT", [P, NMAX], f32, kind="ExternalInput").ap()
    wcat = nc.dram_tensor("wcat", [D, D], f32, kind="ExternalInput").ap()
    bcomb = nc.dram_tensor("bcomb", [1, BF], f32, kind="ExternalInput").ap()
    idx = nc.dram_tensor("idx", [P, S16TOT], mybir.dt.int16,
                         kind="ExternalInput").ap()
    kbig = nc.dram_tensor("kbig", [P, ntiles], f32, kind="ExternalInput").ap()
    shift = nc.dram_tensor("shift", [P, ntiles], f32,
                           kind="ExternalInput").ap()
    ind = nc.dram_tensor("ind", [P, ntiles * GPAD], f32,
                         kind="ExternalInput").ap()
    indT = nc.dram_tensor("indT", [GPAD, ntiles * P], f32,
                          kind="ExternalInput").ap()
    gaux = nc.dram_tensor("gaux", [GPAD, 520], f32, kind="ExternalInput").ap()
    h_out = nc.dram_tensor("h", [NMAX, D], f32, kind="ExternalOutput").ap()

    with tile.TileContext(nc) as tc:
        with (
            tc.tile_pool(name="dram", bufs=1, space="DRAM") as dram,
            tc.tile_pool(name="persist", bufs=1) as pp,
            tc.tile_pool(name="work", bufs=3) as wp,
            tc.tile_pool(name="msgp", bufs=2) as mp,
            tc.tile_pool(name="psum", bufs=2, space="PSUM") as psp,
            tc.tile_pool(name="statps", bufs=1, space="PSUM") as stp,
        ):
            bases_slice = dram.tile([NMAX, BF], f32)
            bases_full = dram.tile([NCORES * NMAX, BF], f32)

            # ---- constants / persistent
            wcat_s = pp.tile([D, D], f32)
            nc.sync.dma_start(wcat_s[:], wcat[:])
            bcomb_s = pp.tile([1, BF], f32)
            nc.sync.dma_start(bcomb_s[:], bcomb[:])
            ones1 = pp.tile([1, P], f32)
            nc.vector.memset(ones1[:], 1.0)
            kbig_s = pp.tile([P, ntiles], f32)
            nc.sync.dma_start(kbig_s[:], kbig[:])
            shift_s = pp.tile([P, ntiles], f32)
            nc.sync.dma_start(shift_s[:], shift[:])
            ind_s = pp.tile([P, ntiles * GPAD], f32)
            nc.sync.dma_start(ind_s[:], ind[:])
            gaux_s = pp.tile([GPAD, 520], f32)
            nc.sync.dma_start(gaux_s[:], gaux[:])

            comb_all = pp.tile([P, ntiles * BF], f32)
            h0_all = pp.tile([P, ntiles * D], f32)

            # ---------------- phase A: bases + comb ----------------
            for t in range(ntiles):
                nt = wp.tile([P, P], f32, tag="nt")
                nc.sync.dma_start(nt[:], nodeT[:, t * P:(t + 1) * P])
                ps = psp.tile([P, D], f32, tag="psA")
                nc.tensor.matmul(ps[:], nt[:], wcat_s[:], start=True,
                                 stop=False)
                nc.tensor.matmul(ps[:, BF:], ones1[:], bcomb_s[:],
                                 start=False, stop=True)
                bsh = wp.tile([P, BF], f32, tag="bsh")
                nc.scalar.activation(bsh[:], ps[:, :BF], ACT.Identity,
                                     bias=shift_s[:, t:t + 1], scale=1.0)
                nc.vector.tensor_copy(comb_all[:, t * BF:(t + 1) * BF],
                                      ps[:, BF:])
                nc.sync.dma_start(bases_slice[t * P:(t + 1) * P, :], bsh[:])

            # ---------------- phase B: allgather ----------------
            nc.gpsimd.collective_compute(
                "AllGather", ALU.bypass,
                replica_groups=[list(range(NCORES))],
                ins=[bases_slice.opt()],
                outs=[bases_full.opt()],
            )

            # ---------------- phase C: gather + aggregate + einsum ----
            stats = stp.tile([GPAD, 2 * D], f32)
            qrot = 0
            s16base = 0
            first_mm = True
            for si, s in enumerate(cfg["sched"]):
                assert s["T"] == 1
                tt = s["t0"]
                wtot = s["wtot"]
                idxt = wp.tile([P, s["s16"]], mybir.dt.int16, tag="idxt")
                nc.sync.dma_start(idxt[:],
                                  idx[:, s16base:s16base + s["s16"]])
                msg = mp.tile([P, wtot, BF], f32, tag="msg")
                for (chx, tr, col0, ncols, s16off) in s["calls"]:
                    nc.gpsimd.dma_gather(
                        msg[:, col0:col0 + ncols, :],
                        bases_full[chx * CHUNK:(chx + 1) * CHUNK, :],
                        idxt[:, s16off:s16off + ncols * 8],
                        ncols * P, ncols * P, BF,
                        queue_num=qrot % 4,
                    )
                    qrot += 1
                aggcat = wp.tile([P, 2 * BF], f32, tag="aggcat")
                mv = msg[:].rearrange("p w f -> p f w")
                nc.vector.tensor_reduce(aggcat[:, :BF], mv, axis=AX.X,
                                        op=ALU.add)
                nc.vector.tensor_reduce(aggcat[:, BF:], mv, axis=AX.X,
                                        op=ALU.max)
                # corrections: sum -= k*BIG ; max -= BIG
                nc.vector.tensor_tensor(
                    out=aggcat[:, :BF],
                    in0=aggcat[:, :BF],
                    in1=kbig_s[:, tt:tt + 1].to_broadcast([P, BF]),
                    op=ALU.subtract)
                nc.vector.tensor_scalar_add(aggcat[:, BF:],
                                            aggcat[:, BF:], -BIG)
                prod = wp.tile([P, H, 8, FH], f32, tag="prod")
                cview = comb_all[:, tt * BF:(tt + 1) * BF].rearrange(
                    "p (h k) -> p h k", h=H)
                nc.gpsimd.tensor_tensor(
                    out=prod[:],
                    in0=cview.to_broadcast([P, H, 8, FH]),
                    in1=aggcat[:].rearrange("p (k f) -> p k f", k=8)
                    [:, None, :, :].broadcast_to([P, H, 8, FH]),
                    op=ALU.mult)
                nc.vector.tensor_reduce(
                    h0_all[:, tt * D:(tt + 1) * D],
                    prod[:].rearrange("p h k f -> p h f k"),
                    axis=AX.X, op=ALU.add)
                hsq = wp.tile([P, D], f32, tag="hsq")
                nc.scalar.square(hsq[:], h0_all[:, tt * D:(tt + 1) * D])
                nc.tensor.matmul(
                    stats[:, :D], ind_s[:, tt * GPAD:(tt + 1) * GPAD],
                    h0_all[:, tt * D:(tt + 1) * D],
                    start=first_mm, stop=(tt == ntiles - 1))
                nc.tensor.matmul(
                    stats[:, D:], ind_s[:, tt * GPAD:(tt + 1) * GPAD],
                    hsq[:],
                    start=first_mm, stop=(tt == ntiles - 1))
                first_mm = False
                s16base += s["s16"]

            # ---------------- phase D: per-graph P/Q ----------------
            st = pp.tile([GPAD, 2 * D], f32)
            nc.vector.tensor_copy(st[:], stats[:])
            cntinv = gaux_s[:, 0:1]
            bias_o = gaux_s[:, 1:129]
            ms = gaux_s[:, 129:257]
            gnw = gaux_s[:, 257:385]
            gnb = gaux_s[:, 385:513]
            s1 = st[:, :D]
            s2 = st[:, D:]
            m0 = pp.tile([GPAD, D], f32)
            nc.vector.tensor_scalar_mul(m0[:], s1, cntinv)
            mh = pp.tile([GPAD, D], f32)
            nc.vector.tensor_tensor(out=mh[:], in0=m0[:], in1=bias_o,
                                    op=ALU.add)
            t1 = pp.tile([GPAD, D], f32)
            nc.vector.scalar_tensor_tensor(out=t1[:], in0=m0[:], scalar=2.0,
                                           in1=bias_o, op0=ALU.mult,
                                           op1=ALU.add)
            t2 = pp.tile([GPAD, D], f32)
            nc.vector.tensor_tensor(out=t2[:], in0=bias_o, in1=t1[:],
                                    op=ALU.mult)
            e2 = pp.tile([GPAD, D], f32)
            nc.vector.tensor_scalar_mul(e2[:], s2, cntinv)
            nc.vector.tensor_tensor(out=e2[:], in0=e2[:], in1=t2[:],
                                    op=ALU.add)
            c0 = pp.tile([GPAD, D], f32)
            nc.vector.tensor_tensor(out=c0[:], in0=mh[:], in1=ms,
                                    op=ALU.mult)
            t3 = pp.tile([GPAD, D], f32)
            nc.vector.tensor_tensor(out=t3[:], in0=c0[:], in1=mh[:],
                                    op=ALU.mult)
            var = pp.tile([GPAD, D], f32)
            nc.vector.scalar_tensor_tensor(out=var[:], in0=t3[:],
                                           scalar=-2.0, in1=e2[:],
                                           op0=ALU.mult, op1=ALU.add)
            t4 = pp.tile([GPAD, D], f32)
            nc.vector.tensor_tensor(out=t4[:], in0=c0[:], in1=c0[:],
                                    op=ALU.mult)
            nc.vector.tensor_tensor(out=var[:], in0=var[:], in1=t4[:],
                                    op=ALU.add)
            stdv = pp.tile([GPAD, D], f32)
            epsc = pp.tile([GPAD, 1], f32)
            nc.vector.memset(epsc[:], EPS)
            nc.scalar.activation(stdv[:], var[:], ACT.Sqrt, bias=epsc[:],
                                 scale=1.0)
            rstd = pp.tile([GPAD, D], f32)
            nc.vector.reciprocal(rstd[:], stdv[:])
            PQ = pp.tile([GPAD, 2 * D], f32)
            nc.vector.tensor_tensor(out=PQ[:, :D], in0=gnw, in1=rstd[:],
                                    op=ALU.mult)
            t5 = pp.tile([GPAD, D], f32)
            nc.vector.tensor_tensor(out=t5[:], in0=bias_o, in1=c0[:],
                                    op=ALU.subtract)
            nc.vector.tensor_tensor(out=PQ[:, D:], in0=t5[:], in1=PQ[:, :D],
                                    op=ALU.mult)
            nc.vector.tensor_tensor(out=PQ[:, D:], in0=PQ[:, D:], in1=gnb,
                                    op=ALU.add)

            # ---------------- phase E: normalize + relu + out ----------
            for t in range(ntiles):
                indt_t = wp.tile([GPAD, P], f32, tag="indt")
                nc.sync.dma_start(indt_t[:], indT[:, t * P:(t + 1) * P])
                pq = psp.tile([P, 2 * D], f32, tag="pq")
                nc.tensor.matmul(pq[:], indt_t[:], PQ[:],
                                 start=True, stop=True)
                hf = wp.tile([P, D], f32, tag="hf")
                nc.vector.tensor_tensor(out=hf[:],
                                        in0=h0_all[:, t * D:(t + 1) * D],
                                        in1=pq[:, :D], op=ALU.mult)
                nc.vector.tensor_tensor(out=hf[:], in0=hf[:], in1=pq[:, D:],
                                        op=ALU.add)
                ho = wp.tile([P, D], f32, tag="ho")
                nc.scalar.activation(ho[:], hf[:], ACT.Relu)
                nc.sync.dma_start(h_out[t * P:(t + 1) * P, :], ho[:])

    nc.compile()
    return nc


_CACHE = {}


def kernel(node, edge_index, edge_attr, batch_ptr, W_bases, W_comb, b_comb,
           bias_out, gn_weight, gn_bias, gn_mean_scale):
    node = np.asarray(node)
    edge_index = np.asarray(edge_index)
    batch_ptr = np.asarray(batch_ptr)
    cfg = _prep(edge_index, batch_ptr)
    in_maps = _make_inputs(cfg, node, W_bases, W_comb, b_comb, bias_out,
                           gn_weight, gn_bias, gn_mean_scale)

    if os.environ.get("EGC_NUMPY_SIM"):
        return _assemble(cfg, _numpy_sim(cfg, in_maps))

    from concourse.bass_utils import run_bass_kernel_spmd
    key = "prog"
    if key not in _CACHE:
        _CACHE[key] = _build(cfg)
    nc = _CACHE[key]
    res = run_bass_kernel_spmd(nc, in_maps, core_ids=list(range(NCORES)),
                               **_CACHE.get("run_kwargs", {}))
    _CACHE["last_res"] = res
    return _assemble(cfg, [res.results[c]["h"] for c in range(NCORES)])


# revision 7
# speedup vs baseline: 1.7078x; 1.7078x over previous
"""EGConv + GraphNorm + ReLU Trainium2 kernel (8 NeuronCores, SPMD).

Strategy (hardcoded for N=100000, E=3200000, D=128, H=8, B=4, A=['sum','max'],
G=64 graphs):
  - Nodes partitioned across 8 cores at graph boundaries (GraphNorm stays
    core-local). Each core owns its dst nodes and their incident edges.
  - Edges gathered per-dst via SWDGE dma_gather (int16 indices -> the padded
    global bases table is split into 4 chunks of 2*NMAX <= 32768 rows; the
    gather source AP per call is one chunk).
  - Per-core dst nodes are sorted by their per-chunk in-degree vector so that
    128-node tiles have tight per-chunk max degrees (less padding).
  - The gather table holds bases + BIG (BIG=512) for real rows and 0 for pad
    rows; padding slots point at pad rows, so
        segment_sum = reduce_sum - k_dst*BIG,   segment_max = reduce_max - BIG.
  - comb/bases matmuls, GraphNorm segment stats (via indicator matmuls) and
    the per-graph affine run on TensorE; reductions and the (comb x aggr)
    einsum run on VectorE/GpSimd.
"""

import math
import os
import numpy as np

# ---------------- problem constants (hardcoded per spec) ----------------
N = 100000
E = 3200000
D = 128
H = 8
NB = 4          # num bases
FH = 16         # per-head dim
G = 64          # graphs
EPS = 1e-5
P = 128
NCORES = 8
BF = 64         # bases feature dim = NB*FH
BIG = 512.0
GPAD = 16       # padded per-core graph count
COLCAP = 8      # max gather columns per call (8*128 = 1024 descriptors)
SLOTCAP = 128   # max T*Wtot slots per supertile (SBUF budget)
TMAX = 1

_F32 = np.float32


def _ceil(a, b):
    return -(-a // b) * b


# ======================================================================
# host preprocessing
# ======================================================================
def _prep(edge_index, batch_ptr):
    counts = np.bincount(batch_ptr, minlength=G).astype(np.int64)
    gcum = np.concatenate([[0], np.cumsum(counts)])  # [G+1]

    # core boundaries at graph boundaries, close to N/8 multiples
    gb = [0]
    for c in range(1, NCORES):
        tgt = N * c / NCORES
        g = int(np.argmin(np.abs(gcum - tgt)))
        g = min(max(g, gb[-1]), G - (NCORES - c))
        gb.append(g)
    gb.append(G)
    node0 = np.array([gcum[gb[c]] for c in range(NCORES)], np.int64)
    ncs = np.array([gcum[gb[c + 1]] - gcum[gb[c]] for c in range(NCORES)],
                   np.int64)
    NMAX = _ceil(int(ncs.max()) + 1, P)
    assert 2 * NMAX <= 32768, (NMAX, ncs.max())
    CHUNK = 2 * NMAX
    ntiles = NMAX // P

    src_g = np.asarray(edge_index[0], np.int64)
    dst_g = np.asarray(edge_index[1], np.int64)
    bounds = np.concatenate([node0, [N]])
    node_core = np.searchsorted(bounds, np.arange(N), side="right") - 1
    node_local = np.arange(N) - node0[node_core]

    src_core = node_core[src_g]
    dst_core = node_core[dst_g]

    cores = []
    for c in range(NCORES):
        n_c = int(ncs[c])
        m = dst_core == c
        es = src_g[m]
        ed = dst_g[m] - node0[c]
        # self loops
        own = np.arange(n_c, dtype=np.int64)
        es = np.concatenate([es, own + node0[c]])
        ed = np.concatenate([ed, own])
        ch = node_core[es] >> 1  # chunk of each edge's src

        # per-(dst, chunk) counts
        kmat = np.bincount(ed * 4 + ch, minlength=n_c * 4).reshape(n_c, 4)
        # sort desc by (max_c k_c, k0, k1, k2, k3) — tight per-tile windows
        kmx = kmat.max(axis=1)
        order = np.lexsort((-kmat[:, 3], -kmat[:, 2], -kmat[:, 1],
                            -kmat[:, 0], -kmx))
        pos_of = np.empty(n_c, np.int64)
        pos_of[order] = np.arange(n_c)
        cores.append(dict(n=n_c, node0=int(node0[c]), perm=order,
                          pos_of=pos_of, es=es, ed=ed, ch=ch, kmat=kmat,
                          g0=gb[c], g1=gb[c + 1]))

    # pid of a global node id: core*NMAX + sorted position
    pid_of = np.empty(N, np.int64)
    for c in range(NCORES):
        cc = cores[c]
        pid_of[cc["node0"]:cc["node0"] + cc["n"]] = c * NMAX + cc["pos_of"]

    # per-core sorted-space per-chunk CSR + k arrays
    for c in range(NCORES):
        cc = cores[c]
        dpos = cc["pos_of"][cc["ed"]]
        key = dpos * 4 + cc["ch"]
        eorder = np.argsort(key, kind="stable")
        rel = (pid_of[cc["es"]] - cc["ch"] * CHUNK)[eorder]
        assert rel.min() >= 0 and rel.max() < CHUNK
        kflat = np.bincount(key, minlength=NMAX * 4)
        indptr = np.concatenate([[0], np.cumsum(kflat)])
        ks = kflat.reshape(NMAX, 4)  # sorted space, dummies are 0
        cc["csr_data"] = rel.astype(np.int64)
        cc["csr_ptr"] = indptr
        cc["ks"] = ks

    # zero-row (pad row) per chunk: core 2c's first pad row
    zrow_rel = np.array([cores[2 * c]["n"] for c in range(4)], np.int64)

    # shared per-tile per-chunk window widths (max over cores)
    Wct = np.zeros((ntiles, 4), np.int64)
    for c in range(NCORES):
        km = cores[c]["ks"].reshape(ntiles, P, 4)
        Wct = np.maximum(Wct, km.max(axis=1))

    # supertile schedule
    sched = []
    t = 0
    while t < ntiles:
        T = 1
        while T < TMAX and t + T < ntiles:
            wc = Wct[t:t + T + 1].max(axis=0)
            if (T + 1) * int(wc.sum()) > SLOTCAP:
                break
            T += 1
        wc = Wct[t:t + T].max(axis=0)
        wtot = int(wc.sum())
        if wtot == 0:
            wtot = 1  # degenerate; shouldn't happen (self loops)
        coff = np.concatenate([[0], np.cumsum(wc)])
        calls = []  # (chunk, tile_rel, col0_in_msg, ncols, s16_off)
        s16 = 0
        for tr in range(T):
            for chx in range(4):
                w = int(wc[chx])
                done = 0
                while done < w:
                    nc_ = min(COLCAP, w - done)
                    calls.append((chx, tr, int(coff[chx]) + done, nc_, s16))
                    s16 += nc_ * 8
                    done += nc_
        sched.append(dict(t0=t, T=T, wc=wc.copy(), wtot=wtot,
                          coff=coff.copy(), calls=calls, s16=s16))
        t += T
    S16TOT = sum(s["s16"] for s in sched)

    # per-core device input arrays
    for c in range(NCORES):
        cc = cores[c]
        data, ptr, ks = cc["csr_data"], cc["csr_ptr"], cc["ks"]
        idx16 = np.zeros((P, S16TOT), np.int16)
        s16base = 0
        for s in sched:
            for (chx, tr, col0, ncols, s16off) in s["calls"]:
                tt = s["t0"] + tr
                dp = tt * P + np.arange(P)
                cnt = ks[dp, chx]
                start = ptr[dp * 4 + chx]
                j0 = col0 - int(s["coff"][chx])
                jj = j0 + np.arange(ncols)[None, :]
                gidx = np.minimum(start[:, None] + jj,
                                  max(len(data) - 1, 0))
                vals = data[gidx] if len(data) else np.zeros((P, ncols),
                                                             np.int64)
                mat = np.where(jj < cnt[:, None], vals, zrow_rel[chx])
                flat = mat.T.reshape(-1)  # [ncols*128], i = col*128 + p
                wrapped = flat.reshape(-1, 16).T.astype(np.int16)  # [16, S]
                idx16[:, s16base + s16off:
                      s16base + s16off + ncols * 8] = np.tile(wrapped, (8, 1))
            s16base += s["s16"]
        cc["idx16"] = idx16

        ktot = ks.sum(axis=1).astype(_F32)  # [NMAX] sorted space
        cc["kbig"] = (ktot.reshape(ntiles, P).T * BIG).astype(_F32)  # [P,nt]
        real = (np.arange(NMAX) < cc["n"])
        cc["shift"] = np.where(real.reshape(ntiles, P).T, _F32(BIG),
                               _F32(0.0)).astype(_F32)

        # graph id per sorted position
        gid = np.full(NMAX, -1, np.int64)
        gnode = np.searchsorted(gcum, cc["node0"] + cc["perm"],
                                side="right") - 1 - cc["g0"]
        gid[:cc["n"]] = gnode
        ind = np.zeros((P, ntiles * GPAD), _F32)
        indT = np.zeros((GPAD, ntiles * P), _F32)
        for tt in range(ntiles):
            gl = gid[tt * P:(tt + 1) * P]
            valid = gl >= 0
            pidx = np.arange(P)[valid]
            gv = gl[valid]
            ind[pidx, tt * GPAD + gv] = 1.0
            indT[gv, tt * P + pidx] = 1.0
        cc["ind"] = ind
        cc["indT"] = indT
        cc["gid"] = gid

        cnt_loc = counts[cc["g0"]:cc["g1"]].astype(_F32)
        cntinv = np.zeros(GPAD, _F32)
        cntinv[:len(cnt_loc)] = 1.0 / np.maximum(cnt_loc, 1.0)
        cc["cntinv"] = cntinv

    return dict(cores=cores, NMAX=NMAX, CHUNK=CHUNK, ntiles=ntiles,
                sched=sched, S16TOT=S16TOT, zrow_rel=zrow_rel,
                node0=node0, ncs=ncs)


def _make_inputs(cfg, node, W_bases, W_comb, b_comb, bias_out, gn_weight,
                 gn_bias, gn_mean_scale):
    node = np.asarray(node, _F32)
    NMAX, ntiles = cfg["NMAX"], cfg["ntiles"]
    wcat = np.concatenate([np.asarray(W_bases, _F32),
                           np.asarray(W_comb, _F32)], axis=1)  # [128,128]
    bcomb = np.asarray(b_comb, _F32).reshape(1, BF)
    gaux = np.zeros((GPAD, 520), _F32)
    gaux[:, 1:129] = np.asarray(bias_out, _F32)[None, :]
    gaux[:, 129:257] = np.asarray(gn_mean_scale, _F32)[None, :]
    gaux[:, 257:385] = np.asarray(gn_weight, _F32)[None, :]
    gaux[:, 385:513] = np.asarray(gn_bias, _F32)[None, :]

    in_maps = []
    for c in range(NCORES):
        cc = cfg["cores"][c]
        nperm = np.zeros((NMAX, D), _F32)
        nperm[:cc["n"]] = node[cc["node0"]:cc["node0"] + cc["n"]][cc["perm"]]
        ga = gaux.copy()
        ga[:, 0] = cc["cntinv"]
        in_maps.append({
            "nodeT": np.ascontiguousarray(nperm.T),        # [128, NMAX]
            "wcat": wcat,
            "bcomb": bcomb,
            "idx": cc["idx16"],                            # [128, S16TOT]
            "kbig": np.ascontiguousarray(cc["kbig"]),      # [128, ntiles]
            "shift": np.ascontiguousarray(cc["shift"]),    # [128, ntiles]
            "ind": np.ascontiguousarray(
                cc["ind"].astype(np.dtype("bfloat16")
                                 if False else _F32)),     # [128, nt*16]
            "indT": np.ascontiguousarray(cc["indT"]),      # [16, nt*128]
            "gaux": ga,                                    # [16, 520]
        })
    return in_maps


# ======================================================================
# numpy simulation of the device algorithm (bit-approximate, for testing)
# ======================================================================
def _numpy_sim(cfg, in_maps):
    NMAX, CHUNK, ntiles = cfg["NMAX"], cfg["CHUNK"], cfg["ntiles"]
    # phase A+B: bases table (shared), per-core comb
    table = np.zeros((NCORES * NMAX, BF), _F32)
    combs = []
    for c in range(NCORES):
        im = in_maps[c]
        nodeT = im["nodeT"]
        full = nodeT.T @ im["wcat"]  # [NMAX, 128]
        bases = full[:, :BF]
        comb = full[:, BF:] + im["bcomb"][0][None, :]
        shift = im["shift"].T.reshape(-1)  # [NMAX]
        table[c * NMAX:(c + 1) * NMAX] = bases + shift[:, None]
        combs.append(comb)

    outs = []
    for c in range(NCORES):
        im = in_maps[c]
        h0 = np.zeros((NMAX, D), _F32)
        s16base = 0
        kbig = im["kbig"].T  # [ntiles, 128]
        for s in cfg["sched"]:
            T, wtot = s["T"], s["wtot"]
            msg = np.zeros((P, T, wtot, BF), _F32)
            for (chx, tr, col0, ncols, s16off) in s["calls"]:
                blk = im["idx"][:16, s16base + s16off:
                                s16base + s16off + ncols * 8]
                # unwrap: value[i] = blk[i % 16, i // 16]
                f2 = blk.T.reshape(-1)  # order (s, p): f2[s*16+p] = blk[p, s]
                vals = f2[:ncols * 128].astype(np.int64)
                rows = table[chx * CHUNK + vals.reshape(ncols, P)]
                msg[:, tr, col0:col0 + ncols, :] = rows.transpose(1, 0, 2)
            s16base += s["s16"]
            ssum = msg.sum(axis=2)                    # [P, T, 64]
            smax = msg.max(axis=2) - _F32(BIG)        # [P, T, 64]
            for tr in range(s["T"]):
                tt = s["t0"] + tr
                su = ssum[:, tr, :] - kbig[tt][:, None]
                aggcat = np.concatenate([su, smax[:, tr, :]], axis=1)
                comb = combs[c][tt * P:(tt + 1) * P]  # [128, 64]
                prod = (comb.reshape(P, H, 8, 1) *
                        aggcat.reshape(P, 1, 8, FH))
                h0[tt * P:(tt + 1) * P] = prod.sum(axis=2).reshape(P, D)
        # graphnorm
        ind = im["ind"].reshape(P, ntiles, GPAD)
        ga = im["gaux"]
        cntinv = ga[:, 0:1]
        bias_o = ga[:, 1:129]
        ms = ga[:, 129:257]
        gnw = ga[:, 257:385]
        gnb = ga[:, 385:513]
        s1 = np.zeros((GPAD, D), _F32)
        s2 = np.zeros((GPAD, D), _F32)
        for tt in range(ntiles):
            ht = h0[tt * P:(tt + 1) * P]
            s1 += ind[:, tt, :].T @ ht
            s2 += ind[:, tt, :].T @ (ht * ht)
        m0 = s1 * cntinv
        mh = m0 + bias_o
        e2 = s2 * cntinv + bias_o * (2 * m0 + bias_o)
        c0 = mh * ms
        var = e2 - 2 * c0 * mh + c0 * c0
        rstd = 1.0 / np.sqrt(var + EPS)
        Pm = gnw * rstd
        Qm = (bias_o - c0) * Pm + gnb
        indT = im["indT"].reshape(GPAD, ntiles, P)
        hfin = np.zeros((NMAX, D), _F32)
        for tt in range(ntiles):
            Pn = indT[:, tt, :].T @ Pm
            Qn = indT[:, tt, :].T @ Qm
            hfin[tt * P:(tt + 1) * P] = np.maximum(
                h0[tt * P:(tt + 1) * P] * Pn + Qn, 0.0)
        outs.append(hfin)
    return outs


def _assemble(cfg, per_core_h):
    out = np.zeros((N, D), _F32)
    for c in range(NCORES):
        cc = cfg["cores"][c]
        out[cc["node0"] + cc["perm"]] = per_core_h[c][:cc["n"]]
    return out


# ======================================================================
# device program
# ======================================================================
def _build(cfg):
    import concourse.bacc as bacc
    import concourse.tile as tile
    from concourse import mybir

    NMAX, CHUNK, ntiles = cfg["NMAX"], cfg["CHUNK"], cfg["ntiles"]
    S16TOT = cfg["S16TOT"]
    f32 = mybir.dt.float32
    bf16 = mybir.dt.bfloat16
    ALU = mybir.AluOpType
    ACT = mybir.ActivationFunctionType
    AX = mybir.AxisListType

    nc = bacc.Bacc("TRN2", target_bir_lowering=False, debug=False,
                   num_devices=NCORES, num_swdge_queues=4)

    nodeT = nc.dram_tensor("nodeT", [P, NMAX], f32, kind="ExternalInput").ap()
    wcat = nc.dram_tensor("wcat", [D, D], f32, kind="ExternalInput").ap()
    bcomb = nc.dram_tensor("bcomb", [1, BF], f32, kind="ExternalInput").ap()
    idx = nc.dram_tensor("idx", [P, S16TOT], mybir.dt.int16,
                         kind="ExternalInput").ap()
    kbig = nc.dram_tensor("kbig", [P, ntiles], f32, kind="ExternalInput").ap()
    shift = nc.dram_tensor("shift", [P, ntiles], f32,
                           kind="ExternalInput").ap()
    ind = nc.dram_tensor("ind", [P, ntiles * GPAD], f32,
                         kind="ExternalInput").ap()
    indT = nc.dram_tensor("indT", [GPAD, ntiles * P], f32,
                          kind="ExternalInput").ap()
    gaux = nc.dram_tensor("gaux", [GPAD, 520], f32, kind="ExternalInput").ap()
    h_out = nc.dram_tensor("h", [NMAX, D], f32, kind="ExternalOutput").ap()

    with tile.TileContext(nc) as tc:
        with (
            tc.tile_pool(name="dram", bufs=1, space="DRAM") as dram,
            tc.tile_pool(name="persist", bufs=1) as pp,
            tc.tile_pool(name="work", bufs=3) as wp,
            tc.tile_pool(name="msgp", bufs=2) as mp,
            tc.tile_pool(name="psum", bufs=2, space="PSUM") as psp,
            tc.tile_pool(name="statps", bufs=1, space="PSUM") as stp,
        ):
            bases_slice = dram.tile([NMAX, BF], f32)
            bases_full = dram.tile([NCORES * NMAX, BF], f32)

            # ---- constants / persistent
            wcat_s = pp.tile([D, D], f32)
            nc.sync.dma_start(wcat_s[:], wcat[:])
            bcomb_s = pp.tile([1, BF], f32)
            nc.sync.dma_start(bcomb_s[:], bcomb[:])
            ones1 = pp.tile([1, P], f32)
            nc.vector.memset(ones1[:], 1.0)
            kbig_s = pp.tile([P, ntiles], f32)
            nc.sync.dma_start(kbig_s[:], kbig[:])
            shift_s = pp.tile([P, ntiles], f32)
            nc.sync.dma_start(shift_s[:], shift[:])
            ind_s = pp.tile([P, ntiles * GPAD], f32)
            nc.sync.dma_start(ind_s[:], ind[:])
            gaux_s = pp.tile([GPAD, 520], f32)
            nc.sync.dma_start(gaux_s[:], gaux[:])

            comb_all = pp.tile([P, ntiles * BF], f32)
            h0_all = pp.tile([P, ntiles * D], f32)

            # ---------------- phase A: bases + comb ----------------
            for t in range(ntiles):
                nt = wp.tile([P, P], f32, tag="nt")
                nc.sync.dma_start(nt[:], nodeT[:, t * P:(t + 1) * P])
                ps = psp.tile([P, D], f32, tag="psA")
                nc.tensor.matmul(ps[:], nt[:], wcat_s[:], start=True,
                                 stop=False)
                nc.tensor.matmul(ps[:, BF:], ones1[:], bcomb_s[:],
                                 start=False, stop=True)
                bsh = wp.tile([P, BF], f32, tag="bsh")
                nc.scalar.activation(bsh[:], ps[:, :BF], ACT.Identity,
                                     bias=shift_s[:, t:t + 1], scale=1.0)
                nc.vector.tensor_copy(comb_all[:, t * BF:(t + 1) * BF],
                                      ps[:, BF:])
                nc.sync.dma_start(bases_slice[t * P:(t + 1) * P, :], bsh[:])

            # ---------------- phase B: allgather ----------------
            nc.gpsimd.collective_compute(
                "AllGather", ALU.bypass,
                replica_groups=[list(range(NCORES))],
                ins=[bases_slice.opt()],
                outs=[bases_full.opt()],
            )

            # ---------------- phase C: gather + aggregate + einsum ----
            stats = stp.tile([GPAD, 2 * D], f32)
            qrot = 0
            s16base = 0
            first_mm = True
            for si, s in enumerate(cfg["sched"]):
                assert s["T"] == 1
                tt = s["t0"]
                wtot = s["wtot"]
                idxt = wp.tile([P, s["s16"]], mybir.dt.int16, tag="idxt")
                nc.sync.dma_start(idxt[:],
                                  idx[:, s16base:s16base + s["s16"]])
                msg = mp.tile([P, wtot, BF], f32, tag="msg")
                for (chx, tr, col0, ncols, s16off) in s["calls"]:
                    nc.gpsimd.dma_gather(
                        msg[:, col0:col0 + ncols, :],
                        bases_full[chx * CHUNK:(chx + 1) * CHUNK, :],
                        idxt[:, s16off:s16off + ncols * 8],
                        ncols * P, ncols * P, BF,
                        queue_num=qrot % 4,
                    )
                    qrot += 1
                aggcat = wp.tile([P, 2 * BF], f32, tag="aggcat")
                mv = msg[:].rearrange("p w f -> p f w")
                nc.vector.tensor_reduce(aggcat[:, :BF], mv, axis=AX.X,
                                        op=ALU.add)
                nc.vector.tensor_reduce(aggcat[:, BF:], mv, axis=AX.X,
                                        op=ALU.max)
                # corrections: sum -= k*BIG ; max -= BIG
                nc.vector.tensor_tensor(
                    out=aggcat[:, :BF],
                    in0=aggcat[:, :BF],
                    in1=kbig_s[:, tt:tt + 1].to_broadcast([P, BF]),
                    op=ALU.subtract)
                nc.vector.tensor_scalar_add(aggcat[:, BF:],
                                            aggcat[:, BF:], -BIG)
                prod = wp.tile([P, H, 8, FH], f32, tag="prod")
                cview = comb_all[:, tt * BF:(tt + 1) * BF].rearrange(
                    "p (h k) -> p h k", h=H)
                nc.vector.tensor_tensor(
                    out=prod[:],
                    in0=cview.to_broadcast([P, H, 8, FH]),
                    in1=aggcat[:].rearrange("p (k f) -> p k f", k=8)
                    [:, None, :, :].broadcast_to([P, H, 8, FH]),
                    op=ALU.mult)
                nc.vector.tensor_reduce(
                    h0_all[:, tt * D:(tt + 1) * D],
                    prod[:].rearrange("p h k f -> p h f k"),
                    axis=AX.X, op=ALU.add)
                hsq = wp.tile([P, D], f32, tag="hsq")
                nc.scalar.square(hsq[:], h0_all[:, tt * D:(tt + 1) * D])
                nc.tensor.matmul(
                    stats[:, :D], ind_s[:, tt * GPAD:(tt + 1) * GPAD],
                    h0_all[:, tt * D:(tt + 1) * D],
                    start=first_mm, stop=(tt == ntiles - 1))
                nc.tensor.matmul(
                    stats[:, D:], ind_s[:, tt * GPAD:(tt + 1) * GPAD],
                    hsq[:],
                    start=first_mm, stop=(tt == ntiles - 1))
                first_mm = False
                s16base += s["s16"]

            # ---------------- phase D: per-graph P/Q ----------------
            st = pp.tile([GPAD, 2 * D], f32)
            nc.vector.tensor_copy(st[:], stats[:])
            cntinv = gaux_s[:, 0:1]
            bias_o = gaux_s[:, 1:129]
            ms = gaux_s[:, 129:257]
            gnw = gaux_s[:, 257:385]
            gnb = gaux_s[:, 385:513]
            s1 = st[:, :D]
            s2 = st[:, D:]
            m0 = pp.tile([GPAD, D], f32)
            nc.vector.tensor_scalar_mul(m0[:], s1, cntinv)
            mh = pp.tile([GPAD, D], f32)
            nc.vector.tensor_tensor(out=mh[:], in0=m0[:], in1=bias_o,
                                    op=ALU.add)
            t1 = pp.tile([GPAD, D], f32)
            nc.vector.scalar_tensor_tensor(out=t1[:], in0=m0[:], scalar=2.0,
                                           in1=bias_o, op0=ALU.mult,
                                           op1=ALU.add)
            t2 = pp.tile([GPAD, D], f32)
            nc.vector.tensor_tensor(out=t2[:], in0=bias_o, in1=t1[:],
                                    op=ALU.mult)
            e2 = pp.tile([GPAD, D], f32)
            nc.vector.tensor_scalar_mul(e2[:], s2, cntinv)
            nc.vector.tensor_tensor(out=e2[:], in0=e2[:], in1=t2[:],
                                    op=ALU.add)
            c0 = pp.tile([GPAD, D], f32)
            nc.vector.tensor_tensor(out=c0[:], in0=mh[:], in1=ms,
                                    op=ALU.mult)
            t3 = pp.tile([GPAD, D], f32)
            nc.vector.tensor_tensor(out=t3[:], in0=c0[:], in1=mh[:],
                                    op=ALU.mult)
            var = pp.tile([GPAD, D], f32)
            nc.vector.scalar_tensor_tensor(out=var[:], in0=t3[:],
                                           scalar=-2.0, in1=e2[:],
                                           op0=ALU.mult, op1=ALU.add)
            t4 = pp.tile([GPAD, D], f32)
            nc.vector.tensor_tensor(out=t4[:], in0=c0[:], in1=c0[:],
                                    op=ALU.mult)
            nc.vector.tensor_tensor(out=var[:], in0=var[:], in1=t4[:],
                                    op=ALU.add)
            stdv = pp.tile([GPAD, D], f32)
            epsc = pp.tile([GPAD, 1], f32)
            nc.vector.memset(epsc[:], EPS)
            nc.scalar.activation(stdv[:], var[:], ACT.Sqrt, bias=epsc[:],
                                 scale=1.0)
            rstd = pp.tile([GPAD, D], f32)
            nc.vector.reciprocal(rstd[:], stdv[:])
            PQ = pp.tile([GPAD, 2 * D], f32)
            nc.vector.tensor_tensor(out=PQ[:, :D], in0=gnw, in1=rstd[:],
                                    op=ALU.mult)
            t5 = pp.tile([GPAD, D], f32)
            nc.vector.tensor_tensor(out=t5[:], in0=bias_o, in1=c0[:],
                                    op=ALU.subtract)
            nc.vector.tensor_tensor(out=PQ[:, D:], in0=t5[:], in1=PQ[:, :D],
                                    op=ALU.mult)
            nc.vector.tensor_tensor(out=PQ[:, D:], in0=PQ[:, D:], in1=gnb,
                                    op=ALU.add)

            # ---------------- phase E: normalize + relu + out ----------
            for t in range(ntiles):
                indt_t = wp.tile([GPAD, P], f32, tag="indt")
                nc.sync.dma_start(indt_t[:], indT[:, t * P:(t + 1) * P])
                pq = psp.tile([P, 2 * D], f32, tag="pq")
                nc.tensor.matmul(pq[:], indt_t[:], PQ[:],
                                 start=True, stop=True)
                hf = wp.tile([P, D], f32, tag="hf")
                nc.vector.tensor_tensor(out=hf[:],
                                        in0=h0_all[:, t * D:(t + 1) * D],
                                        in1=pq[:, :D], op=ALU.mult)
                nc.vector.tensor_tensor(out=hf[:], in0=hf[:], in1=pq[:, D:],
                                        op=ALU.add)
                ho = wp.tile([P, D], f32, tag="ho")
                nc.scalar.activation(ho[:], hf[:], ACT.Relu)
                nc.sync.dma_start(h_out[t * P:(t + 1) * P, :], ho[:])

    nc.compile()
    return nc


_CACHE = {}


def kernel(node, edge_index, edge_attr, batch_ptr, W_bases, W_comb, b_comb,
           bias_out, gn_weight, gn_bias, gn_mean_scale):
    node = np.asarray(node)
    edge_index = np.asarray(edge_index)
    batch_ptr = np.asarray(batch_ptr)
    cfg = _prep(edge_index, batch_ptr)
    in_maps = _make_inputs(cfg, node, W_bases, W_comb, b_comb, bias_out,
                           gn_weight, gn_bias, gn_mean_scale)

    if os.environ.get("EGC_NUMPY_SIM"):
        return _assemble(cfg, _numpy_sim(cfg, in_maps))

    from concourse.bass_utils import run_bass_kernel_spmd
    key = "prog"
    if key not in _CACHE:
        _CACHE[key] = _build(cfg)
    nc = _CACHE[key]
    res = run_bass_kernel_spmd(nc, in_maps, core_ids=list(range(NCORES)),
                               **_CACHE.get("run_kwargs", {}))
    _CACHE["last_res"] = res
    return _assemble(cfg, [res.results[c]["h"] for c in range(NCORES)])
